# revision 25
# baseline (speedup 1.0000x reference)
"""Trainium2 Bass kernel for nn_HeatmapEncoder.

Math per (b, s, c) and per coordinate set (gaze, hand):
    g = exp(-((gx-cx)^2 + (gy-cy)^2) / (2 sigma^2))   on a 336x336 grid
    g = g / (sum(g) + eps)            (zeroed when cx+cy <= 0)
    unified = g_gaze + g_hand
    out = unified / (max(unified) + eps)

Each unified map is rank-2 (separable Gaussian), generated once by three
K=6 bf16 matmuls (hi/lo split of each fp32 factor; yl*xl dropped).
Sum-normalization is folded into the y factors.  Per map the PSUM tile is
drained by ACT (fp16 copy to SBUF) while DVE computes a stride-2 max
(peak underestimate <= 0.5%, well inside the 2e-2 tolerance).  Peaks are
cross-partition-reduced on GPSIMD per group of 4 maps, the scale pass
runs on DVE in fp16 (fast mode), and the output DMA moves 4 maps at a
time as 8 KB descriptors.  Output DRAM layout is partition-major
[112, 32, 1008] fp16; the host reassembles to [32, 336, 336] fp32.

Factor prep computes both x and y factors in one fused [128, 336] chain
(y rows 0-63, x rows 64-127), bounces through DRAM into the 32-aligned
6-row matmul layout, with gather DMAs spread over 4 queues.  A dummy
GPSIMD partition_all_reduce at kernel start pre-loads the Q7 library so
the first real reduce does not stall the pipeline.

Sharding: pure data parallel over batch B=8 across the 8 cores.
"""

import functools
from contextlib import ExitStack

import numpy as np

try:
    import concourse.bass as bass
except ImportError:  # pragma: no cover
    import sys

    sys.path.insert(0, "/opt/trn_rl_repo")
    import concourse.bass as bass

import concourse.tile as tile
from concourse import bacc, bass_isa, mybir
from concourse.bass_utils import run_bass_kernel_spmd

H = W = 336
P = 112  # partitions per y-chunk; y = 3*p + c  (c in 0..2)
NCH = 3
S_DIM, C_DIM = 8, 4
NMAPS = S_DIM * C_DIM  # 32 maps per core
NR = 2 * NMAPS  # 64 coordinate rows (map-major, gaze/hand interleaved)
NB = 8  # free blocks in the aligned factor layout (map j = 4*b + q)
N_CORES = 8
SIGMA = 10.0 / 336.0
EXP_SCALE = -1.0 / (2.0 * SIGMA * SIGMA)
EPS = 1e-6
GROUP = 4

F32 = mybir.dt.float32
BF16 = mybir.dt.bfloat16
FP16 = mybir.dt.float16
FP8 = mybir.dt.float8e4
AF = mybir.ActivationFunctionType
ALU = mybir.AluOpType
AX = mybir.AxisListType

# fp8 pre-scales keep both factor hi/lo terms in e4m3 normal range;
# the drain compensates with an exact power-of-two activation scale
Y_GAIN, X_GAIN = 16.0, 256.0
DRAIN_SCALE = 1.0 / (Y_GAIN * X_GAIN)

GATHER_Q = ("sync", "scalar", "gpsimd")


def _emit(nc, tc, ctx, negc_in, out_t, grid_const, gain_const, stg):
    const = ctx.enter_context(tc.tile_pool(name="const", bufs=1))
    fact = ctx.enter_context(tc.tile_pool(name="fact", bufs=1))
    ffac = ctx.enter_context(tc.tile_pool(name="ffac", bufs=1))
    small = ctx.enter_context(tc.tile_pool(name="small", bufs=2))
    ustage = ctx.enter_context(tc.tile_pool(name="ustage", bufs=8))
    sstage = ctx.enter_context(tc.tile_pool(name="sstage", bufs=2))
    pmap = ctx.enter_context(tc.tile_pool(name="pmap", bufs=2, space="PSUM"))

    # ---- engine warmups: ACT exp table + GPSIMD allreduce library ----
    dum = small.tile([1, 16], F32, tag="dum")
    nc.gpsimd.memset(dum[:], 0.0)
    dum2 = small.tile([1, 16], F32, tag="dum2")
    nc.scalar.activation(dum2[:], dum[:], AF.Exp, bias=0.0, scale=1.0)
    wsrc = small.tile([P, GROUP], F32, tag="wsrc")
    nc.gpsimd.memset(wsrc[:], 0.0)
    wdst = small.tile([P, GROUP], F32, tag="par")
    nc.gpsimd.partition_all_reduce(wdst[:], wsrc[:], channels=P,
                                   reduce_op=bass_isa.ReduceOp.max)

    # ---- constants / inputs (parallel queues) ----
    G = const.tile([2 * NR, W], F32)
    nc.sync.dma_start(G[:], grid_const.ap())
    NC = const.tile([2 * NR, 1], F32)
    nc.scalar.dma_start(NC[:], negc_in.ap())
    GAINC = const.tile([2 * NR, 1], F32)
    nc.sync.dma_start(GAINC[:], gain_const.ap())

    # ---- fused 1-D factor chain: y rows 0-63, x rows 64-127 ----
    sq = fact.tile([2 * NR, W], F32)
    nc.scalar.activation(sq[:], G[:], AF.Square, bias=NC[:, 0:1], scale=1.0)
    f = fact.tile([2 * NR, W], F32)
    nc.scalar.activation(f[:], sq[:], AF.Exp, bias=0.0, scale=EXP_SCALE)

    # per-side normalization 1/(S+delta): the product equals
    # 1/(Sx*Sy + eps) to ~1e-9 relative.  Validity is folded in on the
    # host (invalid rows get cy = -10 so the y factor is exactly 0).
    s = small.tile([2 * NR, 1], F32, tag="s")
    nc.vector.reduce_sum(s[:], f[:], axis=AX.X)
    sd = small.tile([2 * NR, 1], F32, tag="sd")
    nc.vector.tensor_scalar_add(sd[:], s[:], 1e-9)
    srec = small.tile([2 * NR, 1], F32, tag="srec")
    nc.vector.reciprocal(srec[:], sd[:])
    sg = small.tile([2 * NR, 1], F32, tag="sg")
    nc.vector.tensor_mul(sg[:], srec[:], GAINC[:, 0:1])
    nc.vector.tensor_scalar_mul(f[:], f[:], sg[:, 0:1])

    # hi/lo split of both sides in one [128, W] pair of ops
    FH = fact.tile([2 * NR, W], FP8)
    nc.vector.tensor_copy(FH[:], f[:])
    FL = fact.tile([2 * NR, W], FP8)
    nc.vector.tensor_sub(FL[:], f[:], FH[:])

    # ---- bounce through DRAM into the 32-aligned 6-row layout ----
    # plane u rows 0-63 = y-term u (yh, yh, yl); rows 64-127 = x-term
    # (xh, xl, xh).  plane0 == FH so it is a single full DMA.
    sa = stg.ap()  # [3, 128, W]
    nc.sync.dma_start(sa[0], FH[:])
    nc.scalar.dma_start(sa[1][0:NR], FH[0:NR, :])
    nc.scalar.dma_start(sa[1][NR:2 * NR], FL[NR:2 * NR, :])
    nc.sync.dma_start(sa[2][0:NR], FL[0:NR, :])
    nc.sync.dma_start(sa[2][NR:2 * NR], FH[NR:2 * NR, :])

    # per-q factor tiles so map j only depends on its own q's gathers.
    # DoubleRow layout: term u on partition 32q+u, set t in the pair slot.
    FYq = [ffac.tile([128, NB, 2, W], FP8, name=f"FY{q}", tag=f"fy{q}")
           for q in range(4)]
    FXq = [ffac.tile([128, NB, 2, W], FP8, name=f"FX{q}", tag=f"fx{q}")
           for q in range(4)]
    for q in range(4):
        for t in range(2):
            r0 = 2 * q + t
            yq = getattr(nc, GATHER_Q[(2 * q + t) % 3])
            xq = getattr(nc, GATHER_Q[(2 * q + t + 1) % 3])
            # dest [part 32q+u, b, t, x]  <-  stg[u, 8b+2q+t(+64 for x), x]
            yq.dma_start(FYq[q][32 * q:32 * q + 3, :, t, :],
                         sa[:, r0:NR:8, :])
            xq.dma_start(FXq[q][32 * q:32 * q + 3, :, t, :],
                         sa[:, NR + r0:2 * NR:8, :])

    def map_matmuls(j, pt):
        q, b = j % 4, j // 4
        rhs = FXq[q][32 * q:32 * q + 3, b, :, :]
        for cix in range(NCH):
            lhsT = FYq[q][32 * q:32 * q + 3, b, :, cix::3]
            nc.tensor.matmul(pt[:, cix * 512:cix * 512 + W], lhsT, rhs,
                             start=True, stop=True,
                             perf_mode=mybir.MatmulPerfMode.DoubleRow,
                             tile_position=(32 * q, 0))

    uts = {}

    def emit_gen(j):
        """matmuls + fp16 PSUM drain (ACT) + stride-2 max (DVE)."""
        pt = pmap.tile([P, NCH * 512], F32, tag="pmap")
        map_matmuls(j, pt)
        pview = pt[:].rearrange("p (c z) -> p c z", c=NCH)[:, :, 0:W]
        ut = ustage.tile([P, NCH * W], FP16, tag="ust")
        uview = ut[:].rearrange("p (c x) -> p c x", c=NCH)
        nc.scalar.activation(uview, pview, AF.Copy, bias=0.0,
                             scale=DRAIN_SCALE)
        g = j % GROUP
        if g == 0:
            uts["mb"] = small.tile([P, GROUP], F32, name="mb", tag="mb")
        nc.vector.reduce_max(uts["mb"][:, g:g + 1], pview[:, :, 0:W:2],
                             axis=AX.XY)
        uts[j] = ut

    def emit_scale(j0):
        """per-group peak chain + fp16 scale pass + output DMA."""
        mb = uts.pop(("mbv", j0))
        par = small.tile([P, GROUP], F32, tag="par")
        nc.gpsimd.partition_all_reduce(par[:], mb[:], channels=P,
                                       reduce_op=bass_isa.ReduceOp.max)
        pke = small.tile([P, GROUP], F32, tag="pke")
        # PSUM maxes carry the fp8 gain; undo it before adding eps
        nc.vector.tensor_scalar(pke[:], par[:], DRAIN_SCALE, EPS,
                                op0=ALU.mult, op1=ALU.add)
        rg = small.tile([P, GROUP], F32, tag="rg")
        nc.vector.reciprocal(rg[:], pke[:])
        st = sstage.tile([P, GROUP, NCH * W], FP16, tag="sst")
        for j in range(j0, j0 + GROUP):
            g = j - j0
            nc.vector.tensor_scalar_mul(st[:, g, :], uts[j][:],
                                        rg[:, g:g + 1])
            del uts[j]
        nc.sync.dma_start(out_t.ap()[:, j0:j0 + GROUP, :], st[:])

    # software-pipelined emission: group g+1's generation precedes group
    # g's scale phase so stalled scale ops never block the DVE queue head
    for j in range(0, GROUP):
        emit_gen(j)
    uts[("mbv", 0)] = uts.pop("mb")
    for j0 in range(GROUP, NMAPS, GROUP):
        for j in range(j0, j0 + GROUP):
            emit_gen(j)
        uts[("mbv", j0)] = uts.pop("mb")
        emit_scale(j0 - GROUP)
    emit_scale(NMAPS - GROUP)


@functools.lru_cache(maxsize=1)
def _build():
    nc = bacc.Bacc("TRN2", target_bir_lowering=False, debug=False)
    negc_in = nc.dram_tensor("negc", [2 * NR, 1], F32, kind="ExternalInput")
    out_t = nc.dram_tensor("out", [P, NMAPS, NCH * W], FP16,
                           kind="ExternalOutput")

    grid = (np.arange(W, dtype=np.float64) / (W - 1)).astype(np.float32)
    grid_const = nc.inline_tensor(np.tile(grid, (2 * NR, 1)), name="gridc")
    gains = np.full((2 * NR, 1), Y_GAIN, dtype=np.float32)
    gains[NR:] = X_GAIN
    gain_const = nc.inline_tensor(gains, name="gainc")

    stg = nc.dram_tensor("stg", [3, 2 * NR, W], FP8)

    with tile.TileContext(nc) as tc, ExitStack() as ctx:
        _emit(nc, tc, ctx, negc_in, out_t, grid_const, gain_const, stg)
    nc.compile()
    return nc


def _in_map_for(gaze, hand, b):
    cg = np.asarray(gaze[b], dtype=np.float32).reshape(NMAPS, 2)
    ch = np.asarray(hand[b], dtype=np.float32).reshape(NMAPS, 2)
    inter = np.stack([cg, ch], axis=1).reshape(NR, 2)  # row 2*j + t
    negc = np.empty((2 * NR, 1), dtype=np.float32)
    negc[0:NR, 0] = -inter[:, 1]  # y side
    negc[NR:2 * NR, 0] = -inter[:, 0]  # x side
    invalid = ~(inter.sum(axis=1) > 0)
    negc[0:NR, 0][invalid] = 10.0  # kills the y factor -> zero map
    return {"negc": negc}


def kernel(gaze_coords, hand_coords, _trace=False, **trace_kwargs):
    gaze_coords = np.asarray(gaze_coords, dtype=np.float32)
    hand_coords = np.asarray(hand_coords, dtype=np.float32)
    B = gaze_coords.shape[0]
    assert B == N_CORES, f"expected batch {N_CORES}, got {B}"
    nc = _build()
    in_maps = [_in_map_for(gaze_coords, hand_coords, b) for b in range(B)]
    res = run_bass_kernel_spmd(nc, in_maps, list(range(N_CORES)),
                               trace=_trace, **trace_kwargs)
    # device layout [112, 32, 1008] fp16 -> [32, 336, 336] fp32 per core
    outs = []
    for i in range(B):
        arr = np.asarray(res.results[i]["out"]).astype(np.float32)
        arr = arr.reshape(P, NMAPS, NCH, W).transpose(1, 0, 2, 3)
        outs.append(arr.reshape(NMAPS, H, W))
    out = np.stack(outs, axis=0).reshape(B, S_DIM, C_DIM, H, W)
    if _trace:
        return out, res
    return out


# revision 29
# speedup vs baseline: 1.2220x; 1.2220x over previous
"""Trainium2 Bass kernel for nn_HeatmapEncoder.

Math per (b, s, c) and per coordinate set (gaze, hand):
    g = exp(-((gx-cx)^2 + (gy-cy)^2) / (2 sigma^2))   on a 336x336 grid
    g = g / (sum(g) + eps)            (zeroed when cx+cy <= 0)
    unified = g_gaze + g_hand
    out = unified / (max(unified) + eps)

Each unified map is rank-2 (separable Gaussian), generated once by three
K=6 bf16 matmuls (hi/lo split of each fp32 factor; yl*xl dropped).
Sum-normalization is folded into the y factors.  Per map the PSUM tile is
drained by ACT (fp16 copy to SBUF) while DVE computes a stride-2 max
(peak underestimate <= 0.5%, well inside the 2e-2 tolerance).  Peaks are
cross-partition-reduced on GPSIMD per group of 4 maps, the scale pass
runs on DVE in fp16 (fast mode), and the output DMA moves 4 maps at a
time as 8 KB descriptors.  Output DRAM layout is partition-major
[112, 32, 1008] fp16; the host reassembles to [32, 336, 336] fp32.

Factor prep computes both x and y factors in one fused [128, 336] chain
(y rows 0-63, x rows 64-127), bounces through DRAM into the 32-aligned
6-row matmul layout, with gather DMAs spread over 4 queues.  A dummy
GPSIMD partition_all_reduce at kernel start pre-loads the Q7 library so
the first real reduce does not stall the pipeline.

Sharding: pure data parallel over batch B=8 across the 8 cores.
"""

import functools
from contextlib import ExitStack

import numpy as np

try:
    import concourse.bass as bass
except ImportError:  # pragma: no cover
    import sys

    sys.path.insert(0, "/opt/trn_rl_repo")
    import concourse.bass as bass

import concourse.tile as tile
from concourse import bacc, bass_isa, mybir
from concourse.bass_utils import run_bass_kernel_spmd

H = W = 336
P = 112  # partitions per y-chunk; y = 3*p + c  (c in 0..2)
NCH = 3
S_DIM, C_DIM = 8, 4
NMAPS = S_DIM * C_DIM  # 32 maps per core
NR = 2 * NMAPS  # 64 coordinate rows (map-major, gaze/hand interleaved)
NB = 8  # free blocks in the aligned factor layout (map j = 4*b + q)
N_CORES = 8
SIGMA = 10.0 / 336.0
EXP_SCALE = -1.0 / (2.0 * SIGMA * SIGMA)
EPS = 1e-6
GROUP = 4

F32 = mybir.dt.float32
BF16 = mybir.dt.bfloat16
FP16 = mybir.dt.float16
FP8 = mybir.dt.float8e4
AF = mybir.ActivationFunctionType
ALU = mybir.AluOpType
AX = mybir.AxisListType

# fp8 pre-scales keep both factor hi/lo terms in e4m3 normal range;
# the drain compensates with an exact power-of-two activation scale
Y_GAIN, X_GAIN = 16.0, 256.0
DRAIN_SCALE = 1.0 / (Y_GAIN * X_GAIN)

GATHER_Q = ("sync", "scalar", "gpsimd")


def _emit(nc, tc, ctx, negc_in, out_t, grid_const, gain_const, stg):
    const = ctx.enter_context(tc.tile_pool(name="const", bufs=1))
    fact = ctx.enter_context(tc.tile_pool(name="fact", bufs=1))
    ffac = ctx.enter_context(tc.tile_pool(name="ffac", bufs=1))
    small = ctx.enter_context(tc.tile_pool(name="small", bufs=2))
    ustage = ctx.enter_context(tc.tile_pool(name="ustage", bufs=8))
    sstage = ctx.enter_context(tc.tile_pool(name="sstage", bufs=2))
    pmap = ctx.enter_context(tc.tile_pool(name="pmap", bufs=4, space="PSUM"))

    # ---- engine warmups: ACT exp table + GPSIMD allreduce library ----
    dum = small.tile([1, 16], F32, tag="dum")
    nc.gpsimd.memset(dum[:], 0.0)
    dum2 = small.tile([1, 16], F32, tag="dum2")
    nc.scalar.activation(dum2[:], dum[:], AF.Exp, bias=0.0, scale=1.0)
    wsrc = small.tile([P, GROUP], F32, tag="wsrc")
    nc.gpsimd.memset(wsrc[:], 0.0)
    wdst = small.tile([P, GROUP], F32, tag="par")
    nc.gpsimd.partition_all_reduce(wdst[:], wsrc[:], channels=P,
                                   reduce_op=bass_isa.ReduceOp.max)

    # ---- constants / inputs (parallel queues) ----
    G = const.tile([2 * NR, W], F32)
    nc.sync.dma_start(G[:], grid_const.ap())
    NC = const.tile([2 * NR, 1], F32)
    nc.scalar.dma_start(NC[:], negc_in.ap())
    GAINC = const.tile([2 * NR, 1], F32)
    nc.sync.dma_start(GAINC[:], gain_const.ap())

    # ---- fused 1-D factor chain: y rows 0-63, x rows 64-127 ----
    sq = fact.tile([2 * NR, W], F32)
    nc.scalar.activation(sq[:], G[:], AF.Square, bias=NC[:, 0:1], scale=1.0)
    f = fact.tile([2 * NR, W], F32)
    nc.scalar.activation(f[:], sq[:], AF.Exp, bias=0.0, scale=EXP_SCALE)

    # per-side normalization 1/(S+delta): the product equals
    # 1/(Sx*Sy + eps) to ~1e-9 relative.  Validity is folded in on the
    # host (invalid rows get cy = -10 so the y factor is exactly 0).
    s = small.tile([2 * NR, 1], F32, tag="s")
    nc.vector.reduce_sum(s[:], f[:], axis=AX.X)
    sd = small.tile([2 * NR, 1], F32, tag="sd")
    nc.vector.tensor_scalar_add(sd[:], s[:], 1e-9)
    srec = small.tile([2 * NR, 1], F32, tag="srec")
    nc.vector.reciprocal(srec[:], sd[:])
    sg = small.tile([2 * NR, 1], F32, tag="sg")
    nc.vector.tensor_mul(sg[:], srec[:], GAINC[:, 0:1])
    nc.vector.tensor_scalar_mul(f[:], f[:], sg[:, 0:1])

    # hi/lo split of both sides in one [128, W] pair of ops
    FH = fact.tile([2 * NR, W], FP8)
    nc.vector.tensor_copy(FH[:], f[:])
    FL = fact.tile([2 * NR, W], FP8)
    nc.vector.tensor_sub(FL[:], f[:], FH[:])

    # ---- bounce through DRAM into the 32-aligned 6-row layout ----
    # plane u rows 0-63 = y-term u (yh, yh, yl); rows 64-127 = x-term
    # (xh, xl, xh).  plane0 == FH so it is a single full DMA.
    sa = stg.ap()  # [3, 128, W]
    nc.sync.dma_start(sa[0], FH[:])
    nc.scalar.dma_start(sa[1][0:NR], FH[0:NR, :])
    nc.scalar.dma_start(sa[1][NR:2 * NR], FL[NR:2 * NR, :])
    nc.sync.dma_start(sa[2][0:NR], FL[0:NR, :])
    nc.sync.dma_start(sa[2][NR:2 * NR], FH[NR:2 * NR, :])

    # per-q factor tiles so map j only depends on its own q's gathers.
    # DoubleRow layout: term u on partition 32q+u, set t in the pair slot.
    FYq = [ffac.tile([128, NB, 2, W], FP8, name=f"FY{q}", tag=f"fy{q}")
           for q in range(4)]
    FXq = [ffac.tile([128, NB, 2, W], FP8, name=f"FX{q}", tag=f"fx{q}")
           for q in range(4)]
    for q in range(4):
        for t in range(2):
            r0 = 2 * q + t
            yq = getattr(nc, GATHER_Q[(2 * q + t) % 3])
            xq = getattr(nc, GATHER_Q[(2 * q + t + 1) % 3])
            # dest [part 32q+u, b, t, x]  <-  stg[u, 8b+2q+t(+64 for x), x]
            yq.dma_start(FYq[q][32 * q:32 * q + 3, :, t, :],
                         sa[:, r0:NR:8, :])
            xq.dma_start(FXq[q][32 * q:32 * q + 3, :, t, :],
                         sa[:, NR + r0:2 * NR:8, :])

    # per-channel output column ranges; c1 is split at the 512-column PSUM
    # bank boundary so no matmul output ever spans a bank (2-bank tiles)
    MM_COLS = ((0, 0, W), (336, 0, 176), (512, 176, 336), (672, 0, W))
    MM_CIX = (0, 1, 1, 2)

    def map_matmuls(j, pt):
        q, b = j % 4, j // 4
        for (dst0, s0, s1), cix in zip(MM_COLS, MM_CIX):
            lhsT = FYq[q][32 * q:32 * q + 3, b, :, cix::3]
            rhs = FXq[q][32 * q:32 * q + 3, b, :, s0:s1]
            nc.tensor.matmul(pt[:, dst0:dst0 + (s1 - s0)], lhsT, rhs,
                             start=True, stop=True,
                             perf_mode=mybir.MatmulPerfMode.DoubleRow,
                             tile_position=(32 * q, 0))

    uts = {}

    def emit_gen(j):
        """matmuls + fp16 PSUM drain (ACT) + stride-2 max (DVE)."""
        pt = pmap.tile([P, 1024], F32, tag="pmap")  # exactly 2 PSUM banks
        map_matmuls(j, pt)
        ut = ustage.tile([P, NCH * W], FP16, tag="ust")
        nc.scalar.activation(ut[:], pt[:, 0:NCH * W], AF.Copy, bias=0.0,
                             scale=DRAIN_SCALE)
        g = j % GROUP
        if g == 0:
            uts["mb"] = small.tile([P, GROUP], F32, name="mb", tag="mb")
        nc.vector.reduce_max(uts["mb"][:, g:g + 1], pt[:, 0:NCH * W:2],
                             axis=AX.X)
        uts[j] = ut

    def emit_scale(j0):
        """per-group peak chain + fp16 scale pass + output DMA."""
        mb = uts.pop(("mbv", j0))
        par = small.tile([P, GROUP], F32, tag="par")
        nc.gpsimd.partition_all_reduce(par[:], mb[:], channels=P,
                                       reduce_op=bass_isa.ReduceOp.max)
        pke = small.tile([P, GROUP], F32, tag="pke")
        # PSUM maxes carry the fp8 gain; undo it before adding eps
        nc.vector.tensor_scalar(pke[:], par[:], DRAIN_SCALE, EPS,
                                op0=ALU.mult, op1=ALU.add)
        rg = small.tile([P, GROUP], F32, tag="rg")
        nc.vector.reciprocal(rg[:], pke[:])
        st = sstage.tile([P, GROUP, NCH * W], FP16, tag="sst")
        for j in range(j0, j0 + GROUP):
            g = j - j0
            nc.vector.tensor_scalar_mul(st[:, g, :], uts[j][:],
                                        rg[:, g:g + 1])
            del uts[j]
        nc.sync.dma_start(out_t.ap()[:, j0:j0 + GROUP, :], st[:])

    # software-pipelined emission: group g+1's generation precedes group
    # g's scale phase so stalled scale ops never block the DVE queue head
    for j in range(0, GROUP):
        emit_gen(j)
    uts[("mbv", 0)] = uts.pop("mb")
    for j0 in range(GROUP, NMAPS, GROUP):
        for j in range(j0, j0 + GROUP):
            emit_gen(j)
        uts[("mbv", j0)] = uts.pop("mb")
        emit_scale(j0 - GROUP)
    emit_scale(NMAPS - GROUP)


@functools.lru_cache(maxsize=1)
def _build():
    nc = bacc.Bacc("TRN2", target_bir_lowering=False, debug=False)
    negc_in = nc.dram_tensor("negc", [2 * NR, 1], F32, kind="ExternalInput")
    out_t = nc.dram_tensor("out", [P, NMAPS, NCH * W], FP16,
                           kind="ExternalOutput")

    grid = (np.arange(W, dtype=np.float64) / (W - 1)).astype(np.float32)
    grid_const = nc.inline_tensor(np.tile(grid, (2 * NR, 1)), name="gridc")
    gains = np.full((2 * NR, 1), Y_GAIN, dtype=np.float32)
    gains[NR:] = X_GAIN
    gain_const = nc.inline_tensor(gains, name="gainc")

    stg = nc.dram_tensor("stg", [3, 2 * NR, W], FP8)

    with tile.TileContext(nc) as tc, ExitStack() as ctx:
        _emit(nc, tc, ctx, negc_in, out_t, grid_const, gain_const, stg)
    nc.compile()
    return nc


def _in_map_for(gaze, hand, b):
    cg = np.asarray(gaze[b], dtype=np.float32).reshape(NMAPS, 2)
    ch = np.asarray(hand[b], dtype=np.float32).reshape(NMAPS, 2)
    inter = np.stack([cg, ch], axis=1).reshape(NR, 2)  # row 2*j + t
    negc = np.empty((2 * NR, 1), dtype=np.float32)
    negc[0:NR, 0] = -inter[:, 1]  # y side
    negc[NR:2 * NR, 0] = -inter[:, 0]  # x side
    invalid = ~(inter.sum(axis=1) > 0)
    negc[0:NR, 0][invalid] = 10.0  # kills the y factor -> zero map
    return {"negc": negc}


def kernel(gaze_coords, hand_coords, _trace=False, **trace_kwargs):
    gaze_coords = np.asarray(gaze_coords, dtype=np.float32)
    hand_coords = np.asarray(hand_coords, dtype=np.float32)
    B = gaze_coords.shape[0]
    assert B == N_CORES, f"expected batch {N_CORES}, got {B}"
    nc = _build()
    in_maps = [_in_map_for(gaze_coords, hand_coords, b) for b in range(B)]
    res = run_bass_kernel_spmd(nc, in_maps, list(range(N_CORES)),
                               trace=_trace, **trace_kwargs)
    # device layout [112, 32, 1008] fp16 -> [32, 336, 336] fp32 per core
    outs = []
    for i in range(B):
        arr = np.asarray(res.results[i]["out"]).astype(np.float32)
        arr = arr.reshape(P, NMAPS, NCH, W).transpose(1, 0, 2, 3)
        outs.append(arr.reshape(NMAPS, H, W))
    out = np.stack(outs, axis=0).reshape(B, S_DIM, C_DIM, H, W)
    if _trace:
        return out, res
    return out
